# revision 1
# baseline (speedup 1.0000x reference)
"""Multi-head attention (B=2, S=2048, D=1024, H=16, HD=64) on 8 TRN2 cores.

Sharding (hybrid DP/TP, SPMD one-graph):
  core c: batch b = c//4, head-group g = c%4 (heads 4g..4g+3 of batch b).
  - QKV projections: Megatron column-split (each core its 4 heads).
  - attention: fully local per (batch, head).
  - att outputs (attT layout [hd, S] per head, bf16) AllGather'd per head
    within the 4-core batch group -> every core holds all 16 heads.
  - O-projection: Megatron column-split on wo (each core owns 256 output
    channels for ALL tokens of its batch; the wo column slice is a
    per-core input, so the compiled graph is identical across cores).
  - host gather: pure concat over (batch, output-channel slice).

Layouts on chip:
  xT  [D, S]  (x transposed on host)  -> SBUF [128, 8, 2048] f32r
  qT/kT [heads*hd, S] -> SBUF [128, 2, 2048] f32r (computed via PE)
  v natural [S, 4 heads, 128] bf16 where cols = [v(64) | ones | zeros]
  scoresT [s_k part, s_q free] in PSUM; exp on ACT -> bf16; PV matmul
  -> psum [128, 512] whose row 64 is the softmax denominator.
QK/scores matmuls run as float32r (fp32 storage, 1 cycle/row on PE);
the exp/PV/O-proj path runs bf16 (f32 PSUM accumulation).
"""

import numpy as np
import ml_dtypes

B, S, D = 2, 2048, 1024
H, HD = 16, 64
N_CORES = 8
G = 4                      # cores per batch group
HPC = 4                    # heads per core
CW = HPC * HD              # per-core projection width = 256
ATT_SCALE = float(HD) ** -0.5
P = 128

_CACHED_NC = None


def _build():
    import concourse.mybir as mybir
    import concourse.tile as tile
    from concourse import bacc

    f32 = mybir.dt.float32
    f32r = mybir.dt.float32r
    bf16 = mybir.dt.bfloat16
    Exp = mybir.ActivationFunctionType.Exp
    add = mybir.AluOpType.add
    mult = mybir.AluOpType.mult

    nc = bacc.Bacc("TRN2", target_bir_lowering=False, debug=False,
                   num_devices=N_CORES)

    xT = nc.declare_dram_parameter("xT", [D, S], f32r, isOutput=False)
    wq = nc.declare_dram_parameter("wq", [D, CW], f32r, isOutput=False)
    wk = nc.declare_dram_parameter("wk", [D, CW], f32r, isOutput=False)
    wv = nc.declare_dram_parameter("wv", [D, CW], f32r, isOutput=False)
    bq = nc.declare_dram_parameter("bq", [CW], f32, isOutput=False)
    bk = nc.declare_dram_parameter("bk", [CW], f32, isOutput=False)
    bv = nc.declare_dram_parameter("bv", [CW], f32, isOutput=False)
    wo = nc.declare_dram_parameter("wo", [D, CW], bf16, isOutput=False)
    bo = nc.declare_dram_parameter("bo", [CW], f32, isOutput=False)
    out = nc.declare_dram_parameter("out", [S, CW], f32, isOutput=True)

    groups = [[0, 1, 2, 3], [4, 5, 6, 7]]
    KC = D // P           # 8 contraction chunks
    SC = S // P           # 16 token chunks of 128
    NQ = 512              # moving free dim per matmul
    SQC = S // NQ         # 4 query chunks of 512

    with tile.TileContext(nc) as tc:
        with (
            tc.tile_pool(name="const", bufs=1) as const,
            tc.tile_pool(name="acts", bufs=1) as acts,
            tc.tile_pool(name="exps", bufs=6) as exps,
            tc.tile_pool(name="attw", bufs=4) as attw,
            tc.tile_pool(name="small", bufs=2) as small,
            tc.tile_pool(name="ostage", bufs=3) as ostage,
            tc.tile_pool(name="dram", bufs=1, space="DRAM") as dram,
        ):
            # ---- constant loads -------------------------------------
            xt_sb = const.tile([P, KC, S], f32r)
            for ki in range(KC):
                nc.sync.dma_start(xt_sb[:, ki, :],
                                  xT[ki * P:(ki + 1) * P, :])
            wq_sb = const.tile([P, KC, CW], f32r, tag="wq")
            wk_sb = const.tile([P, KC, CW], f32r, tag="wk")
            wv_sb = const.tile([P, KC, CW], f32r, tag="wv")
            nc.sync.dma_start(wq_sb[:], wq.ap().rearrange("(k p) m -> p k m", p=P))
            nc.sync.dma_start(wk_sb[:], wk.ap().rearrange("(k p) m -> p k m", p=P))
            nc.sync.dma_start(wv_sb[:], wv.ap().rearrange("(k p) m -> p k m", p=P))
            wo_sb = const.tile([P, KC, CW], bf16, tag="wo")
            nc.sync.dma_start(wo_sb[:], wo.ap().rearrange("(k p) n -> p k n", p=P))
            bq_sb = const.tile([P, 2], f32, tag="bq")
            bk_sb = const.tile([P, 2], f32, tag="bk")
            nc.sync.dma_start(bq_sb[:], bq.ap().rearrange("(j p) -> p j", p=P))
            nc.sync.dma_start(bk_sb[:], bk.ap().rearrange("(j p) -> p j", p=P))
            bv_bc = const.tile([P, CW], f32, tag="bv")
            bo_bc = const.tile([P, CW], f32, tag="bo")
            nc.sync.dma_start(bv_bc[:], bv.ap().partition_broadcast(P))
            nc.sync.dma_start(bo_bc[:], bo.ap().partition_broadcast(P))

            ones_f = const.tile([1, HD], f32, tag="onesf")
            ones_r = const.tile([1, HD], f32r, tag="onesr")
            nc.vector.memset(ones_f[:], 1.0)
            with nc.allow_low_precision("f32r is fp32 storage"):
                nc.vector.tensor_copy(ones_r[:], ones_f[:])

            qT_sb = acts.tile([P, 2, S], f32r, tag="qT")
            kT_sb = acts.tile([P, 2, S], f32r, tag="kT")
            # v cols per head: [v(64) | ones(1) | zeros(63)] -> lhsT M=128
            v_sb = acts.tile([P, SC, HPC, P], bf16, tag="v")
            nc.vector.memset(v_sb[:, :, :, HD + 1:], 0.0)
            nc.vector.memset(v_sb[:, :, :, HD:HD + 1], 1.0)

            # ---- projections ----------------------------------------
            with tc.tile_pool(name="pp", bufs=3, space="PSUM") as pp:
                # qT / kT: [128(2 heads*hd), S] = w_chunk.T @ xT
                for (w_sb, b_sb, dst) in ((wq_sb, bq_sb, qT_sb),
                                          (wk_sb, bk_sb, kT_sb)):
                    for j in range(2):
                        for si in range(SQC):
                            ps = pp.tile([P, NQ], f32, tag="pq")
                            for ki in range(KC):
                                nc.tensor.matmul(
                                    ps[:],
                                    w_sb[:, ki, j * P:(j + 1) * P],
                                    xt_sb[:, ki, si * NQ:(si + 1) * NQ],
                                    start=(ki == 0), stop=(ki == KC - 1),
                                )
                            with nc.allow_low_precision("f32r is fp32 storage"):
                                nc.vector.tensor_tensor(
                                    dst[:, j, si * NQ:(si + 1) * NQ], ps[:],
                                    b_sb[:, j:j + 1].to_broadcast((P, NQ)),
                                    add)
                # v natural: [S, 256] = xT_chunk.T @ wv
                for si in range(SC):
                    ps = pp.tile([P, CW], f32, tag="pv")
                    for ki in range(KC):
                        nc.tensor.matmul(
                            ps[:],
                            xt_sb[:, ki, si * P:(si + 1) * P],
                            wv_sb[:, ki, :],
                            start=(ki == 0), stop=(ki == KC - 1),
                        )
                    nc.vector.tensor_tensor(
                        v_sb[:, si, :, :HD],
                        ps.rearrange("p (h x) -> p h x", x=HD),
                        bv_bc.rearrange("p (h x) -> p h x", x=HD), add)

            # ---- attention + per-head AllGather ---------------------
            agin = [dram.tile([HD, S], bf16, tag=f"agin{h}", name=f"agin{h}")
                    for h in range(HPC)]
            agout = [dram.tile([G, HD, S], bf16, tag=f"agout{h}",
                               name=f"agout{h}") for h in range(HPC)]
            GRP = 4  # mi-chunks per batched run (uniform PE runs of 8)
            with (
                tc.tile_pool(name="sc", bufs=3, space="PSUM") as scp,
                tc.tile_pool(name="pv", bufs=2, space="PSUM") as pvp,
            ):
                for h in range(HPC):
                    j2, off = h // 2, (h % 2) * HD
                    for half in range(2):
                        pv_ps = [pvp.tile([P, NQ], f32, tag="pv",
                                          name=f"pv{h}_{half}_{i}")
                                 for i in range(2)]
                        for g0 in range(0, SC, GRP):
                            sct = []
                            for mi in range(g0, g0 + GRP):
                                sc_ps = scp.tile([P, 2 * NQ], f32, tag="sc",
                                                 name=f"sc{h}_{half}_{mi}")
                                sct.append(sc_ps)
                                for qq in range(2):
                                    sq = half * 2 + qq
                                    nc.tensor.matmul(
                                        sc_ps[:, qq * NQ:(qq + 1) * NQ],
                                        kT_sb[off:off + HD, j2,
                                              mi * P:(mi + 1) * P],
                                        qT_sb[off:off + HD, j2,
                                              sq * NQ:(sq + 1) * NQ],
                                        start=True, stop=True,
                                    )
                            ets = []
                            for i, mi in enumerate(range(g0, g0 + GRP)):
                                et = exps.tile([P, 2 * NQ], bf16, tag="exp",
                                               name=f"et{h}_{half}_{mi}")
                                ets.append(et)
                                nc.scalar.activation(et[:], sct[i][:], Exp,
                                                     scale=ATT_SCALE)
                            for i, mi in enumerate(range(g0, g0 + GRP)):
                                for qq in range(2):
                                    nc.tensor.matmul(
                                        pv_ps[qq][:],
                                        v_sb[:, mi, h, :],
                                        ets[i][:, qq * NQ:(qq + 1) * NQ],
                                        start=(mi == 0), stop=(mi == SC - 1),
                                    )
                        # softmax divide + bf16 cast, stage for AllGather
                        for qq in range(2):
                            sq = half * 2 + qq
                            rec = small.tile([1, NQ], f32r, tag="rec")
                            with nc.allow_low_precision("f32r is fp32"):
                                nc.vector.reciprocal(
                                    rec[:], pv_ps[qq][HD:HD + 1, :])
                            rb = scp.tile([HD, NQ], f32, tag="sc",
                                          name=f"rb{h}_{sq}")
                            nc.tensor.matmul(rb[:], ones_r[:], rec[:],
                                             start=True, stop=True)
                            rb_sb = attw.tile([HD, NQ], f32, tag="rbs")
                            nc.vector.tensor_copy(rb_sb[:], rb[:])
                            at = attw.tile([HD, NQ], bf16, tag="att")
                            nc.vector.tensor_tensor(
                                at[:], pv_ps[qq][:HD, :], rb_sb[:], mult)
                            nc.sync.dma_start(
                                agin[h][:, sq * NQ:(sq + 1) * NQ], at[:])
                    nc.gpsimd.collective_compute(
                        "AllGather", mybir.AluOpType.bypass,
                        replica_groups=groups,
                        ins=[agin[h].opt()],
                        outs=[agout[h].opt()],
                    )

            # ---- O-projection (column-sharded, all tokens) ----------
            # kc-outer so each AllGather's contribution starts as soon as it
            # lands; 16 accumulators packed 2-per-PSUM-bank.
            with (
                tc.tile_pool(name="attk", bufs=3) as attk,
                tc.tile_pool(name="op", bufs=8, space="PSUM") as op,
            ):
                for hs in range(2):  # two passes of 8 token-chunks each
                    po = [op.tile([P, CW], f32, tag="po",
                                  name=f"po{hs}_{i}") for i in range(8)]
                    for kc in range(KC):
                        h, jj = kc // 2, kc % 2
                        atk = attk.tile([P, S // 2], bf16, tag="atk",
                                        name=f"atk{hs}_{kc}")
                        nc.sync.dma_start(
                            atk[:],
                            agout[h][2 * jj:2 * jj + 2, :,
                                     hs * (S // 2):(hs + 1) * (S // 2)]
                            .rearrange("a b s -> (a b) s"))
                        for i in range(8):
                            nc.tensor.matmul(
                                po[i][:],
                                atk[:, i * P:(i + 1) * P],
                                wo_sb[:, kc, :],
                                start=(kc == 0), stop=(kc == KC - 1),
                            )
                    for i in range(8):
                        si = hs * 8 + i
                        ot = ostage.tile([P, CW], f32, tag="ot")
                        nc.vector.tensor_tensor(ot[:], po[i][:], bo_bc[:],
                                                add)
                        nc.sync.dma_start(out[si * P:(si + 1) * P, :], ot[:])

    nc.compile()
    return nc


def _get_nc():
    global _CACHED_NC
    if _CACHED_NC is None:
        _CACHED_NC = _build()
    return _CACHED_NC


# permutation of global head index by (ag_h, source core j): head 4j+h
_HEAD_ORDER = [4 * j + h for h in range(HPC) for j in range(G)]


def kernel(x, wq, bq, wk, bk, wv, bv, wo, bo):
    from concourse.bass_utils import run_bass_kernel_spmd

    x = np.asarray(x, dtype=np.float32)
    wq = np.asarray(wq, dtype=np.float32)
    wk = np.asarray(wk, dtype=np.float32)
    wv = np.asarray(wv, dtype=np.float32)
    wo = np.asarray(wo, dtype=np.float32)
    bq = np.asarray(bq, dtype=np.float32)
    bk = np.asarray(bk, dtype=np.float32)
    bv = np.asarray(bv, dtype=np.float32)
    bo = np.asarray(bo, dtype=np.float32)

    nc = _get_nc()

    # wo rows reordered to the (ag_h, source_core) K-chunk order used on chip
    wo_perm = np.ascontiguousarray(
        wo.reshape(H, HD, D)[_HEAD_ORDER].reshape(D, D))

    in_maps = []
    for c in range(N_CORES):
        b, g = c // G, c % G
        cs = slice(g * CW, (g + 1) * CW)
        in_maps.append({
            "xT": np.ascontiguousarray(x[b].T),
            "wq": np.ascontiguousarray(wq[:, cs]),
            "wk": np.ascontiguousarray(wk[:, cs]),
            "wv": np.ascontiguousarray(wv[:, cs]),
            "bq": np.ascontiguousarray(bq[cs]),
            "bk": np.ascontiguousarray(bk[cs]),
            "bv": np.ascontiguousarray(bv[cs]),
            "wo": np.ascontiguousarray(wo_perm[:, cs]).astype(
                ml_dtypes.bfloat16),
            "bo": np.ascontiguousarray(bo[cs]),
        })

    res = run_bass_kernel_spmd(nc, in_maps, core_ids=list(range(N_CORES)))

    full = np.empty((B, S, D), dtype=np.float32)
    for c in range(N_CORES):
        b, g = c // G, c % G
        full[b, :, g * CW:(g + 1) * CW] = res.results[c]["out"]
    return full



# revision 7
# speedup vs baseline: 1.2151x; 1.2151x over previous
"""Multi-head attention (B=2, S=2048, D=1024, H=16, HD=64) on 8 TRN2 cores.

Sharding (hybrid DP/TP, SPMD one-graph):
  core c: batch b = c//4, head-group g = c%4 (heads 4g..4g+3 of batch b).
  - QKV projections: Megatron column-split (each core its 4 heads).
  - attention: fully local per (batch, head).
  - att outputs ([64, S] per head, bf16) AllGather'd per (head, token-half)
    within the 4-core batch group -> every core holds all 16 heads.
  - O-projection: Megatron column-split on wo.
  - host gather: pure concat over (batch, output-channel slice).

All matmul operands bf16 (fp32 PSUM accumulation).  Layout highlights:
  xT [D, S] bf16 -> SBUF [128, 8, 2048]
  qT/kT [128(2 heads*64), 2, S] bf16 computed on PE
  v natural [S, 4 heads, 128] bf16, cols = [v(64) | ones | zeros]
  scoresT [s_k part, s_q free] f32 in PSUM; exp on ACT -> bf16
  PV -> psum [128, 512]: rows 0-63 numerator, row 64 softmax denominator.
Normalization (reciprocal / broadcast / divide) is deferred into the next
head's score loop so the Scalar engine (exp, the bottleneck) never idles.
"""

import numpy as np
import ml_dtypes

B, S, D = 2, 2048, 1024
H, HD = 16, 64
N_CORES = 8
G = 4                      # cores per batch group
HPC = 4                    # heads per core
CW = HPC * HD              # per-core projection width = 256
ATT_SCALE = float(HD) ** -0.5
P = 128
NQ = 512

_CACHED_NC = None


def _build():
    import concourse.mybir as mybir
    import concourse.tile as tile
    from concourse import bacc

    f32 = mybir.dt.float32
    bf16 = mybir.dt.bfloat16
    Exp = mybir.ActivationFunctionType.Exp
    add = mybir.AluOpType.add
    mult = mybir.AluOpType.mult

    nc = bacc.Bacc("TRN2", target_bir_lowering=False, debug=False,
                   num_devices=N_CORES)

    xT = nc.declare_dram_parameter("xT", [D, S], bf16, isOutput=False)
    wq = nc.declare_dram_parameter("wq", [D, CW], bf16, isOutput=False)
    wk = nc.declare_dram_parameter("wk", [D, CW], bf16, isOutput=False)
    wv = nc.declare_dram_parameter("wv", [D, CW], bf16, isOutput=False)
    bq = nc.declare_dram_parameter("bq", [CW], f32, isOutput=False)
    bk = nc.declare_dram_parameter("bk", [CW], f32, isOutput=False)
    bv = nc.declare_dram_parameter("bv", [CW], f32, isOutput=False)
    wo = nc.declare_dram_parameter("wo", [D, CW], bf16, isOutput=False)
    bo = nc.declare_dram_parameter("bo", [CW], f32, isOutput=False)
    out = nc.declare_dram_parameter("out", [S, CW], f32, isOutput=True)

    groups = [[0, 1, 2, 3], [4, 5, 6, 7]]
    KC = D // P           # 8 contraction chunks
    SC = S // P           # 16 key chunks of 128
    SB = 4                # token blocks of 512 for x DMA / projections

    with tile.TileContext(nc) as tc:
        with (
            tc.tile_pool(name="const", bufs=1) as const,
            tc.tile_pool(name="acts", bufs=1) as acts,
            tc.tile_pool(name="exps", bufs=6) as exps,
            tc.tile_pool(name="attw", bufs=2) as attw,
            tc.tile_pool(name="ostage", bufs=3) as ostage,
            tc.tile_pool(name="dram", bufs=1, space="DRAM") as dram,
        ):
            # ---- input DMAs, priority order ---------------------------
            wq_sb = const.tile([P, KC, CW], bf16, tag="wq")
            wk_sb = const.tile([P, KC, CW], bf16, tag="wk")
            nc.sync.dma_start(wq_sb[:], wq.ap().rearrange("(k p) m -> p k m", p=P))
            nc.sync.dma_start(wk_sb[:], wk.ap().rearrange("(k p) m -> p k m", p=P))
            xt_sb = const.tile([P, KC, S], bf16, tag="xt")
            for sb in range(SB):
                for ki in range(KC):
                    nc.sync.dma_start(
                        xt_sb[:, ki, sb * NQ:(sb + 1) * NQ],
                        xT[ki * P:(ki + 1) * P, sb * NQ:(sb + 1) * NQ])
            wv_sb = const.tile([P, KC, CW], bf16, tag="wv")
            nc.sync.dma_start(wv_sb[:], wv.ap().rearrange("(k p) m -> p k m", p=P))
            bq_sb = const.tile([P, 2], f32, tag="bq")
            bk_sb = const.tile([P, 2], f32, tag="bk")
            nc.sync.dma_start(bq_sb[:], bq.ap().rearrange("(j p) -> p j", p=P))
            nc.sync.dma_start(bk_sb[:], bk.ap().rearrange("(j p) -> p j", p=P))
            bv_bc = const.tile([P, CW], f32, tag="bv")
            nc.sync.dma_start(bv_bc[:], bv.ap().partition_broadcast(P))
            wo_sb = const.tile([P, KC, CW], bf16, tag="wo")
            nc.sync.dma_start(wo_sb[:], wo.ap().rearrange("(k p) n -> p k n", p=P))
            bo_bc = const.tile([P, CW], f32, tag="bo")
            nc.sync.dma_start(bo_bc[:], bo.ap().partition_broadcast(P))

            ones_f = const.tile([P, HD], f32, tag="onesf")
            nc.vector.memset(ones_f[:], 1.0)

            qT_sb = acts.tile([P, 2, S], bf16, tag="qT")
            kT_sb = acts.tile([P, 2, S], bf16, tag="kT")
            # v cols per head: [v(64) | ones(1) | zeros(63)] -> lhsT M=128
            v_sb = acts.tile([P, SC, HPC, P], bf16, tag="v")
            nc.vector.memset(v_sb[:, :, :, HD + 1:], 0.0)
            nc.vector.memset(v_sb[:, :, :, HD:HD + 1], 1.0)

            def emit_qk_proj(w_sb, b_sb, dst, j, sb, pool, lbl):
                ps = pool.tile([P, NQ], f32, tag="pq",
                               name=f"pq{lbl}_{j}_{sb}")
                for ki in range(KC):
                    nc.tensor.matmul(
                        ps[:],
                        w_sb[:, ki, j * P:(j + 1) * P],
                        xt_sb[:, ki, sb * NQ:(sb + 1) * NQ],
                        start=(ki == 0), stop=(ki == KC - 1),
                    )
                with nc.allow_low_precision("bf16 activations"):
                    nc.vector.tensor_tensor(
                        dst[:, j, sb * NQ:(sb + 1) * NQ], ps[:],
                        b_sb[:, j:j + 1].to_broadcast((P, NQ)), add)

            def emit_v_proj(si, pool):
                ps = pool.tile([P, CW], f32, tag="pq", name=f"pvv{si}")
                for ki in range(KC):
                    nc.tensor.matmul(
                        ps[:],
                        xt_sb[:, ki, si * P:(si + 1) * P],
                        wv_sb[:, ki, :],
                        start=(ki == 0), stop=(ki == KC - 1),
                    )
                with nc.allow_low_precision("bf16 activations"):
                    nc.vector.tensor_tensor(
                        v_sb[:, si, :, :HD],
                        ps.rearrange("p (h x) -> p h x", x=HD),
                        bv_bc.rearrange("p (h x) -> p h x", x=HD), add)

            # ---- j2=0 projections up front ---------------------------
            with tc.tile_pool(name="pp1", bufs=2, space="PSUM") as pp1:
                for sb in range(SB):
                    emit_qk_proj(wq_sb, bq_sb, qT_sb, 0, sb, pp1, "q")
                for sb in range(SB):
                    emit_qk_proj(wk_sb, bk_sb, kT_sb, 0, sb, pp1, "k")

            # ---- attention --------------------------------------------
            # group order tuned so AGs spread + late proj fits in slack
            GROUP_ORDER = [(0, 0), (0, 1), (1, 0), (1, 1),
                           (2, 0), (3, 0), (2, 1), (3, 1)]
            agin = {}
            agout = {}
            for h in range(HPC):
                for half in range(2):
                    agin[(h, half)] = dram.tile(
                        [HD, 2 * NQ], bf16, tag=f"agi{h}_{half}",
                        name=f"agi{h}_{half}")
                    agout[(h, half)] = dram.tile(
                        [G, HD, 2 * NQ], bf16, tag=f"ago{h}_{half}",
                        name=f"ago{h}_{half}")

            # extra PE work interleaved into each group's score loop,
            # keyed (group_idx, mi) -> list of thunks
            side_work = {}
            with tc.tile_pool(name="pp2", bufs=1, space="PSUM") as pp2:
                for si in range(SC):          # v proj inside group 0
                    side_work.setdefault((0, si), []).append(
                        (lambda si=si: emit_v_proj(si, pp2)))
                for sb in range(SB):          # q j2=1 in groups 1-2
                    gi, mi = (1, 2 + 3 * sb) if sb < 2 else (2, 2 + 3 * (sb - 2))
                    side_work.setdefault((gi, mi), []).append(
                        (lambda sb=sb: emit_qk_proj(
                            wq_sb, bq_sb, qT_sb, 1, sb, pp2, "q")))
                for sb in range(SB):          # k j2=1 in groups 2-3
                    gi, mi = (2, 8 + 3 * sb) if sb < 2 else (3, 2 + 3 * (sb - 2))
                    side_work.setdefault((gi, mi), []).append(
                        (lambda sb=sb: emit_qk_proj(
                            wk_sb, bk_sb, kT_sb, 1, sb, pp2, "k")))

                with (
                    tc.tile_pool(name="scp", bufs=2, space="PSUM") as scp,
                    tc.tile_pool(name="pvp", bufs=2, space="PSUM") as pvp,
                    tc.tile_pool(name="rbp", bufs=1, space="PSUM") as rbp,
                ):
                    pending_norm = [None]

                    def emit_norm():
                        if pending_norm[0] is None:
                            return
                        h, half, pvs, rec = pending_norm[0]
                        pending_norm[0] = None
                        rb = rbp.tile([P, NQ], f32, tag="rb",
                                      name=f"rb{h}_{half}")
                        nc.tensor.matmul(rb[0:HD, :], ones_f[0:1, :],
                                         rec[0:1, :], start=True, stop=True)
                        nc.tensor.matmul(rb[HD:P, :], ones_f[32:33, :],
                                         rec[32:33, :], start=True, stop=True)
                        at = attw.tile([P, NQ], bf16, tag="at")
                        with nc.allow_low_precision("bf16 att"):
                            nc.vector.tensor_tensor(at[:], pvs[:], rb[:], mult)
                        nc.sync.dma_start(
                            agin[(h, half)][:, 0:NQ], at[0:HD, :])
                        nc.sync.dma_start(
                            agin[(h, half)][:, NQ:2 * NQ], at[HD:P, :])
                        nc.gpsimd.collective_compute(
                            "AllGather", mybir.AluOpType.bypass,
                            replica_groups=groups,
                            ins=[agin[(h, half)].opt()],
                            outs=[agout[(h, half)].opt()],
                        )

                    for gi, (h, half) in enumerate(GROUP_ORDER):
                        j2, off = h // 2, (h % 2) * HD
                        pv_ps = [pvp.tile([P, NQ], f32, tag="pv",
                                          name=f"pv{h}_{half}_{qq}")
                                 for qq in range(2)]
                        for mi in range(SC):
                            if mi == 3:
                                emit_norm()
                            sc = scp.tile([P, 2 * NQ], f32, tag="sc",
                                          name=f"sc{h}_{half}_{mi}")
                            for qq in range(2):
                                sq = half * 2 + qq
                                nc.tensor.matmul(
                                    sc[:, qq * NQ:(qq + 1) * NQ],
                                    kT_sb[off:off + HD, j2,
                                          mi * P:(mi + 1) * P],
                                    qT_sb[off:off + HD, j2,
                                          sq * NQ:(sq + 1) * NQ],
                                    start=True, stop=True,
                                )
                            for fn in side_work.pop((gi, mi), ()):
                                fn()
                            et = exps.tile([P, 2 * NQ], bf16, tag="exp",
                                           name=f"et{h}_{half}_{mi}")
                            nc.scalar.activation(et[:], sc[:], Exp,
                                                 scale=ATT_SCALE)
                            for qq in range(2):
                                nc.tensor.matmul(
                                    pv_ps[qq][:],
                                    v_sb[:, mi, h, :],
                                    et[:, qq * NQ:(qq + 1) * NQ],
                                    start=(mi == 0), stop=(mi == SC - 1),
                                )
                        # drain numerators+denominators, free pv banks
                        pvs = attw.tile([P, NQ], f32, tag="pvs")
                        den = attw.tile([33, NQ], f32, tag="den")
                        nc.vector.tensor_copy(pvs[0:HD, :], pv_ps[0][0:HD, :])
                        nc.vector.tensor_copy(pvs[HD:P, :], pv_ps[1][0:HD, :])
                        nc.vector.tensor_copy(den[0:1, :],
                                              pv_ps[0][HD:HD + 1, :])
                        nc.vector.tensor_copy(den[32:33, :],
                                              pv_ps[1][HD:HD + 1, :])
                        rec = attw.tile([33, NQ], f32, tag="rec")
                        nc.vector.reciprocal(rec[:], den[:])
                        pending_norm[0] = (h, half, pvs, rec)
                    emit_norm()

            # ---- O-projection (column-sharded, all tokens) ----------
            with tc.tile_pool(name="op", bufs=8, space="PSUM") as op:
                for hs in range(2):  # two passes of 8 token-chunks each
                    po = [op.tile([P, CW], f32, tag="po",
                                  name=f"po{hs}_{i}") for i in range(8)]
                    with tc.tile_pool(name="attk", bufs=3) as attk:
                        for kc in range(KC):
                            h, jj = kc // 2, kc % 2
                            atk = attk.tile([P, S // 2], bf16, tag="atk",
                                            name=f"atk{hs}_{kc}")
                            nc.sync.dma_start(
                                atk[:],
                                agout[(h, hs)][2 * jj:2 * jj + 2, :, :]
                                .rearrange("a b s -> (a b) s"))
                            for i in range(8):
                                nc.tensor.matmul(
                                    po[i][:],
                                    atk[:, i * P:(i + 1) * P],
                                    wo_sb[:, kc, :],
                                    start=(kc == 0), stop=(kc == KC - 1),
                                )
                        for i in range(8):
                            si = hs * 8 + i
                            ot = ostage.tile([P, CW], f32, tag="ot")
                            nc.vector.tensor_tensor(ot[:], po[i][:],
                                                    bo_bc[:], add)
                            nc.sync.dma_start(out[si * P:(si + 1) * P, :],
                                              ot[:])

    nc.compile()
    return nc


def _get_nc():
    global _CACHED_NC
    if _CACHED_NC is None:
        _CACHED_NC = _build()
    return _CACHED_NC


# permutation of global head index by (ag_h, source core j): head 4j+h
_HEAD_ORDER = [4 * j + h for h in range(HPC) for j in range(G)]


def kernel(x, wq, bq, wk, bk, wv, bv, wo, bo):
    from concourse.bass_utils import run_bass_kernel_spmd

    bf = ml_dtypes.bfloat16
    x = np.asarray(x, dtype=np.float32)
    wq = np.asarray(wq, dtype=np.float32)
    wk = np.asarray(wk, dtype=np.float32)
    wv = np.asarray(wv, dtype=np.float32)
    wo = np.asarray(wo, dtype=np.float32)
    bq = np.asarray(bq, dtype=np.float32)
    bk = np.asarray(bk, dtype=np.float32)
    bv = np.asarray(bv, dtype=np.float32)
    bo = np.asarray(bo, dtype=np.float32)

    nc = _get_nc()

    # wo rows reordered to the (ag_h, source_core) K-chunk order used on chip
    wo_perm = np.ascontiguousarray(
        wo.reshape(H, HD, D)[_HEAD_ORDER].reshape(D, D))

    in_maps = []
    for c in range(N_CORES):
        b, g = c // G, c % G
        cs = slice(g * CW, (g + 1) * CW)
        in_maps.append({
            "xT": np.ascontiguousarray(x[b].T).astype(bf),
            "wq": np.ascontiguousarray(wq[:, cs]).astype(bf),
            "wk": np.ascontiguousarray(wk[:, cs]).astype(bf),
            "wv": np.ascontiguousarray(wv[:, cs]).astype(bf),
            "bq": np.ascontiguousarray(bq[cs]),
            "bk": np.ascontiguousarray(bk[cs]),
            "bv": np.ascontiguousarray(bv[cs]),
            "wo": np.ascontiguousarray(wo_perm[:, cs]).astype(bf),
            "bo": np.ascontiguousarray(bo[cs]),
        })

    res = run_bass_kernel_spmd(nc, in_maps, core_ids=list(range(N_CORES)))

    full = np.empty((B, S, D), dtype=np.float32)
    for c in range(N_CORES):
        b, g = c // G, c % G
        full[b, :, g * CW:(g + 1) * CW] = res.results[c]["out"]
    return full
